# revision 1
# baseline (speedup 1.0000x reference)
"""Trainium2 kernel for nn_CantileverPINN: MLP 1->15->30->60->1 value + first
4 derivatives w.r.t. the scalar input x at N=524288 collocation points.

Strategy: each of the 5 outputs is a smooth scalar function of x on [0,1)
(tanh-MLP composition, analytic; Chebyshev coefficients decay ~10x per 2
terms and reach the fp32 floor by degree 15).  The host computes the exact
derivatives via Taylor-mode propagation at 65 Chebyshev nodes (float64),
fits degree-15 Chebyshev series for the 5 outputs, and the device evaluates
the series at all points:

  theta  = arccos(2x-1)            (via arctan + Newton-refined rsqrt)
  phi    = theta / 2pi             in [0, 0.5]
  q      = k * phi                 (PE outer product, k = 0..15, 8 point-
                                    groups packed per 128-partition tile)
  r      = q - round(q)            (DVE magic-constant rounding)
  basis  = cos(2*pi*r) = sin(pi/2 - 2*pi*|r|)   (ACT Abs + Sin, args in
                                                 [-pi/2, pi/2] where the
                                                 table is ~5e-8 accurate)
  out    = C^T basis               (PE contraction, block-diagonal C)

Data parallel over 8 cores: each core handles 65536 points ([128, 512]
tile); 16 supertiles of 8 point-rows each.  All matmuls fp32.
"""

import numpy as np

_N = 524288
_NCORES = 8
_NPC = _N // _NCORES      # 65536 points per core
_F = 512                  # free-dim columns per tile
_ROWS = _NPC // _F        # 128 point-rows per core
_G = 8                    # point-rows (groups) per supertile
_KB = 16                  # Chebyshev terms per group (degree 15)
_NST = _ROWS // _G        # 16 supertiles
_NORD = 5                 # outputs: w, w_x, w_xx, w_xxx, w_xxxx
_MAGIC = 12582912.0       # 1.5 * 2**23: (q + M) - M == round(q) for |q| < 2**22
_TWO_PI = float(2.0 * np.pi)

_compiled = {}


# ----------------------------------------------------------------- host math
def _taylor_mlp(x, W1, b1, W2, b2, W3, b3, W4, b4):
    """Exact value + derivatives (orders 0..4) of the MLP at points x.

    float64 throughout; returns [5, n]."""
    x = np.asarray(x, np.float64)
    n = x.shape[0]
    W1, b1, W2, b2, W3, b3, W4, b4 = [
        np.asarray(a, np.float64) for a in (W1, b1, W2, b2, W3, b3, W4, b4)
    ]
    w1 = W1[0]
    a0 = x[:, None] * w1[None, :] + b1[None, :]
    a1 = np.broadcast_to(w1[None, :], (n, w1.shape[0])).copy()
    a2 = np.zeros_like(a0)
    a3 = np.zeros_like(a0)
    a4 = np.zeros_like(a0)

    def tanh_chain(a0, a1, a2, a3, a4):
        t = np.tanh(a0)
        u = 1.0 - t * t
        s2 = -2.0 * t * u
        s3 = u * (6.0 * t * t - 2.0)
        s4 = 8.0 * t * u * (2.0 - 3.0 * t * t)
        h0 = t
        h1 = u * a1
        h2 = s2 * a1**2 + u * a2
        h3 = s3 * a1**3 + 3.0 * s2 * a1 * a2 + u * a3
        h4 = (s4 * a1**4 + 6.0 * s3 * a1**2 * a2
              + s2 * (3.0 * a2**2 + 4.0 * a1 * a3) + u * a4)
        return h0, h1, h2, h3, h4

    for W, b in ((W2, b2), (W3, b3)):
        h = tanh_chain(a0, a1, a2, a3, a4)
        a0 = h[0] @ W + b[None, :]
        a1 = h[1] @ W
        a2 = h[2] @ W
        a3 = h[3] @ W
        a4 = h[4] @ W
    h = tanh_chain(a0, a1, a2, a3, a4)
    return np.stack([(h[i] @ W4)[:, 0] + (b4[0] if i == 0 else 0.0)
                     for i in range(5)])


def _fit_chebyshev(W1, b1, W2, b2, W3, b3, W4, b4):
    """Chebyshev coefficients [5, _KB] of the 5 outputs on x in [0,1]."""
    D = 64  # fit degree (Clenshaw-Curtis); truncate to _KB terms
    j = np.arange(D + 1)
    xn = (np.cos(np.pi * j / D) + 1.0) / 2.0
    g = _taylor_mlp(xn, W1, b1, W2, b2, W3, b3, W4, b4)       # [5, D+1]
    km = np.cos(np.pi * np.outer(j, j) / D)
    wts = np.ones(D + 1)
    wts[0] = 0.5
    wts[-1] = 0.5
    c = (2.0 / D) * (g * wts[None, :]) @ km
    c[:, 0] *= 0.5
    c[:, -1] *= 0.5
    return c[:, :_KB]


# ------------------------------------------------------------- device kernel
def _build_program():
    import concourse.bacc as bacc
    import concourse.tile as tile
    from concourse import mybir

    AluOp = mybir.AluOpType
    Act = mybir.ActivationFunctionType
    f32 = mybir.dt.float32

    bf16 = mybir.dt.bfloat16

    nc = bacc.Bacc(trn_type="TRN2", target_bir_lowering=False, debug=False,
                   num_devices=_NCORES)
    x_d = nc.declare_dram_parameter("x", [_ROWS, _F], f32, isOutput=False)
    # outer lhsT: 3 stacked copies of the block-diagonal k matrix (one per
    # phi bf16 part) -> single K=24 bf16 matmul per supertile
    kv_d = nc.declare_dram_parameter("kv", [3 * _G, 128], bf16, isOutput=False)
    cm_d = nc.declare_dram_parameter("cm", [128, _NORD * _G], f32,
                                     isOutput=False)
    gam_d = nc.declare_dram_parameter("gam", [_NORD * _G, 1], f32,
                                      isOutput=False)
    out_d = nc.declare_dram_parameter("out", [_NORD, _NPC], f32, isOutput=True)

    with tile.TileContext(nc) as tc:
        with tc.tile_pool(name="consts", bufs=1) as consts, \
             tc.tile_pool(name="pre", bufs=1) as pre, \
             tc.tile_pool(name="stq", bufs=3, space="PSUM") as stq, \
             tc.tile_pool(name="sto", bufs=3, space="PSUM") as sto, \
             tc.tile_pool(name="stsb", bufs=3) as stsb:
            kv = consts.tile([3 * _G, 128], bf16)
            nc.sync.dma_start(out=kv, in_=kv_d[:, :])
            cm = consts.tile([128, _NORD * _G], f32)
            nc.sync.dma_start(out=cm, in_=cm_d[:, :])
            gam = consts.tile([_NORD * _G, 1], f32)
            nc.sync.dma_start(out=gam, in_=gam_d[:, :])

            # ---- preprocessing: phi = arccos(2x-1) / 2pi, once per core,
            # pipelined in 4 column chunks.  Two phases so each ACT table
            # set (natural_log_exp, then trig_and_small) loads exactly once.
            CF = _F // 4
            xs = pre.tile([_ROWS, _F], f32)
            v = pre.tile([_ROWS, _F], f32)
            v2 = pre.tile([_ROWS, _F], f32)
            s = pre.tile([_ROWS, _F], f32)
            sc = pre.tile([_ROWS, _F], f32)
            lns = pre.tile([_ROWS, _F], f32)
            r0 = pre.tile([_ROWS, _F], f32)
            u = pre.tile([_ROWS, _F], f32)
            at = pre.tile([_ROWS, _F], f32)
            phi = pre.tile([_ROWS, _F], f32)
            ph = pre.tile([_ROWS, _F], bf16)
            t2 = pre.tile([_ROWS, _F], f32)
            pm = pre.tile([_ROWS, _F], bf16)
            t3 = pre.tile([_ROWS, _F], f32)
            pl = pre.tile([_ROWS, _F], bf16)
            # phase A: u = v * rsqrt(1 - v^2) via exp(-0.5 ln s)
            for c in range(4):
                cs = slice(c * CF, (c + 1) * CF)
                nc.sync.dma_start(out=xs[:, cs], in_=x_d[:, cs])
                nc.vector.tensor_scalar(v[:, cs], xs[:, cs], 2.0, -1.0,
                                        AluOp.mult, AluOp.add)
                nc.vector.tensor_mul(v2[:, cs], v[:, cs], v[:, cs])
                nc.vector.tensor_scalar(s[:, cs], v2[:, cs], -1.0, 1.0,
                                        AluOp.mult, AluOp.add)
                nc.vector.tensor_scalar_max(sc[:, cs], s[:, cs], 1e-20)
                nc.scalar.activation(lns[:, cs], sc[:, cs], Act.Ln)
                nc.scalar.activation(r0[:, cs], lns[:, cs], Act.Exp,
                                     scale=-0.5)
                nc.vector.tensor_mul(u[:, cs], v[:, cs], r0[:, cs])
            # phase B: phi = 0.25 - arctan(u)/2pi, then split into 3 bf16
            # parts (k<=15 is exact in bf16; the 3 parts carry 24 mantissa
            # bits, making the bf16 outer product fp32-exact)
            for c in range(4):
                cs = slice(c * CF, (c + 1) * CF)
                nc.scalar.activation(at[:, cs], u[:, cs], Act.Arctan)
                nc.vector.tensor_scalar(phi[:, cs], at[:, cs],
                                        float(-1.0 / _TWO_PI), 0.25,
                                        AluOp.mult, AluOp.add)
                nc.vector.tensor_copy(ph[:, cs], phi[:, cs])
                nc.vector.tensor_sub(t2[:, cs], phi[:, cs], ph[:, cs])
                nc.vector.tensor_copy(pm[:, cs], t2[:, cs])
                nc.vector.tensor_sub(t3[:, cs], t2[:, cs], pm[:, cs])
                nc.vector.tensor_copy(pl[:, cs], t3[:, cs])
            # reshape into one [24, 16*512] tile: part p rows at 8p..8p+7,
            # group g on partitions (matmul rhs must start at partition 0),
            # supertiles along the free dim.  st-major issue order so early
            # supertiles unblock the PE as soon as possible.
            p8 = pre.tile([3 * _G, _NST * _F], bf16)
            for st in range(_NST):
                for pi, ptile in enumerate((ph, pm, pl)):
                    eng = nc.sync if pi == 0 else nc.gpsimd
                    eng.dma_start(
                        out=p8[pi * _G:(pi + 1) * _G,
                               st * _F:(st + 1) * _F],
                        in_=ptile[st * _G:(st + 1) * _G, :])

            out3 = out_d.rearrange("o (r f) -> o r f", f=_F)

            for st in range(_NST):
                lo = st * _F
                hi = (st + 1) * _F
                q_ps = stq.tile([128, _F], f32)
                nc.tensor.matmul(q_ps, lhsT=kv, rhs=p8[:, lo:hi],
                                 start=True, stop=True)
                rnd = stsb.tile([128, _F], f32)
                nc.vector.tensor_scalar(rnd, q_ps, _MAGIC, _MAGIC,
                                        AluOp.add, AluOp.subtract)
                r = stsb.tile([128, _F], f32)
                nc.vector.tensor_sub(r, q_ps, rnd)
                # half-angle: cos(2 pi r) = 1 - 2 sin^2(pi r).  Sin args stay
                # in [-pi/2, pi/2]; the -2 is folded into cm, the +Sum(c_k)
                # into the output copy's bias.
                sn = stsb.tile([128, _F], f32)
                nc.scalar.activation(sn, r, Act.Sin, scale=float(np.pi))
                basis = stsb.tile([128, _F], f32)
                nc.gpsimd.tensor_mul(basis, sn, sn)
                o_ps = sto.tile([_NORD * _G, _F], f32)
                nc.tensor.matmul(o_ps, lhsT=cm, rhs=basis,
                                 start=True, stop=True)
                osb = stsb.tile([_NORD * _G, _F], f32)
                nc.scalar.activation(osb, o_ps, Act.Identity, bias=gam)
                # one DMA per supertile: SBUF side is a plain [40, 512] tile
                # (single partition dim); the DRAM side iterates (o, g, f) in
                # the same o-major order as the tile's partitions
                nc.sync.dma_start(out=out3[:, st * _G:(st + 1) * _G, :],
                                  in_=osb[:, :])

    nc.finalize()
    return nc


def _get_program():
    if "nc" not in _compiled:
        _compiled["nc"] = _build_program()
    return _compiled["nc"]


def _build_kv():
    import ml_dtypes
    kv1 = np.zeros((_G, 128), np.float32)
    for g in range(_G):
        kv1[g, g * _KB:(g + 1) * _KB] = np.arange(_KB, dtype=np.float32)
    return np.vstack([kv1, kv1, kv1]).astype(ml_dtypes.bfloat16)


def _build_cm(c):
    """c: [5, _KB] float64 -> half-angle block lhsT [128, 5*_G] float32 with
    coefficients -2*c_k, plus the gamma bias vector [5*_G, 1] (= sum_k c_k)."""
    cmat = np.zeros((128, _NORD * _G), np.float32)
    gam = np.zeros((_NORD * _G, 1), np.float32)
    for g in range(_G):
        for o in range(_NORD):
            cmat[g * _KB:(g + 1) * _KB, o * _G + g] = \
                (-2.0 * c[o]).astype(np.float32)
            gam[o * _G + g, 0] = np.float32(c[o].sum())
    return cmat, gam


def _run(inputs, **spmd_kwargs):
    """Shard, run on 8 cores, gather. Returns (out [5, N], BassKernelResults)."""
    from concourse.bass_utils import run_bass_kernel_spmd

    x = np.ascontiguousarray(np.asarray(inputs["x"], np.float32))
    assert x.shape == (_N,), f"unexpected x shape {x.shape}"
    c = _fit_chebyshev(inputs["W1"], inputs["b1"], inputs["W2"], inputs["b2"],
                       inputs["W3"], inputs["b3"], inputs["W4"], inputs["b4"])
    kv = _build_kv()
    cm, gam = _build_cm(c)
    nc = _get_program()

    xs = x.reshape(_NCORES, _ROWS, _F)
    in_maps = [{"x": np.ascontiguousarray(xs[i]), "kv": kv, "cm": cm,
                "gam": gam}
               for i in range(_NCORES)]
    res = run_bass_kernel_spmd(nc, in_maps, core_ids=list(range(_NCORES)),
                               **spmd_kwargs)
    out = np.concatenate([res.results[i]["out"] for i in range(_NCORES)],
                         axis=1)
    return np.ascontiguousarray(out.astype(np.float32)), res


def kernel(**inputs):
    out, _ = _run(inputs)
    return out


if __name__ == "__main__":
    rng = np.random.default_rng(0)
    fake = {
        "x": rng.uniform(0, 1, _N).astype(np.float32),
        "W1": (rng.standard_normal((1, 15)) * 0.5).astype(np.float32),
        "b1": np.zeros(15, np.float32),
        "W2": (rng.standard_normal((15, 30)) * 0.25).astype(np.float32),
        "b2": np.zeros(30, np.float32),
        "W3": (rng.standard_normal((30, 60)) * 0.18).astype(np.float32),
        "b3": np.zeros(60, np.float32),
        "W4": (rng.standard_normal((60, 1)) * 0.13).astype(np.float32),
        "b4": np.zeros(1, np.float32),
    }
    out = kernel(**fake)
    ref = _taylor_mlp(fake["x"], fake["W1"], fake["b1"], fake["W2"],
                      fake["b2"], fake["W3"], fake["b3"], fake["W4"],
                      fake["b4"])
    for i in range(5):
        scale = np.abs(ref[i]).max()
        err = np.abs(out[i] - ref[i]).max()
        print(f"order {i}: absmax_err={err:.3e} rel={err / scale:.3e}")



# revision 10
# speedup vs baseline: 1.5426x; 1.5426x over previous
"""Trainium2 kernel for nn_CantileverPINN: MLP 1->15->30->60->1 value + first
4 derivatives w.r.t. the scalar input x at N=524288 collocation points.

Strategy: each of the 5 outputs is a smooth scalar function of x on [0,1)
(tanh-MLP composition, analytic).  Parity-fold about x=0.5: with s = x-0.5
and z = s^2 in [0, 0.25],

    f_o(x) = E_o(z) + s * O_o(z)

where E_o, O_o are degree-7 polynomials in z (equivalent to a degree-15
fit in x; host fits them from exact float64 Taylor-mode derivatives of the
MLP).  The z-monomial basis on [0, 0.25] is well conditioned here because
the Chebyshev coefficients of the outputs decay ~10x per z-degree while
monomial conversion on [0, 0.25] amplifies only ~5.8x per z-degree
(measured evaluation condition number kappa <= 3).

Device evaluation per point:
    z   = s*s + 1e-20          (DVE, exact fp32)
    lnz = Ln(z)                (ACT)
    q   = k * lnz              (PE outer product, k = 0..7 block-diagonal,
                                lnz carried as bf16 hi+lo pair -> exact to
                                ~1e-5; 16 point-groups per 128-partition
                                supertile)
    B_k = Exp(q) = z^k         (ACT)
    oE  = E^T B, oO = O^T B    (PE contraction, block-diagonal coeffs)
    out = oE + s * oO          (s replicated to all 5 outputs by a third
                                PE matmul from its bf16 hi+lo pair)

Data parallel over 8 cores: each core handles 65536 points ([128, 512]
tile); 8 supertiles of 16 point-rows each.
"""

import numpy as np

_N = 524288
_NCORES = 8
_NPC = _N // _NCORES      # 65536 points per core
_F = 512                  # free-dim columns per tile
_ROWS = _NPC // _F        # 128 point-rows per core
_G = 16                   # point-rows (groups) per supertile
_KB = 8                   # z-monomial terms per group (degree 7 in z)
_NST = _ROWS // _G        # 8 supertiles
_NORD = 5                 # outputs: w, w_x, w_xx, w_xxx, w_xxxx
_MROW = _NORD * _G        # 80 output rows per supertile (o-major)

_MM_DTYPE = "float32"     # contraction matmul dtype: "float32" | "float32r"

_compiled = {}


# ----------------------------------------------------------------- host math
def _taylor_mlp(x, W1, b1, W2, b2, W3, b3, W4, b4):
    """Exact value + derivatives (orders 0..4) of the MLP at points x.

    float64 throughout; returns [5, n]."""
    x = np.asarray(x, np.float64)
    n = x.shape[0]
    W1, b1, W2, b2, W3, b3, W4, b4 = [
        np.asarray(a, np.float64) for a in (W1, b1, W2, b2, W3, b3, W4, b4)
    ]
    w1 = W1[0]
    a0 = x[:, None] * w1[None, :] + b1[None, :]
    a1 = np.broadcast_to(w1[None, :], (n, w1.shape[0])).copy()
    a2 = np.zeros_like(a0)
    a3 = np.zeros_like(a0)
    a4 = np.zeros_like(a0)

    def tanh_chain(a0, a1, a2, a3, a4):
        t = np.tanh(a0)
        u = 1.0 - t * t
        s2 = -2.0 * t * u
        s3 = u * (6.0 * t * t - 2.0)
        s4 = 8.0 * t * u * (2.0 - 3.0 * t * t)
        h0 = t
        h1 = u * a1
        h2 = s2 * a1**2 + u * a2
        h3 = s3 * a1**3 + 3.0 * s2 * a1 * a2 + u * a3
        h4 = (s4 * a1**4 + 6.0 * s3 * a1**2 * a2
              + s2 * (3.0 * a2**2 + 4.0 * a1 * a3) + u * a4)
        return h0, h1, h2, h3, h4

    for W, b in ((W2, b2), (W3, b3)):
        h = tanh_chain(a0, a1, a2, a3, a4)
        a0 = h[0] @ W + b[None, :]
        a1 = h[1] @ W
        a2 = h[2] @ W
        a3 = h[3] @ W
        a4 = h[4] @ W
    h = tanh_chain(a0, a1, a2, a3, a4)
    return np.stack([(h[i] @ W4)[:, 0] + (b4[0] if i == 0 else 0.0)
                     for i in range(5)])


def _fit_even_odd(W1, b1, W2, b2, W3, b3, W4, b4):
    """Monomial coefficients [5, _KB] of E_o(z), O_o(z) on z in [0, 0.25]
    where f_o(0.5 + s) = E_o(s^2) + s * O_o(s^2)."""
    from numpy.polynomial import chebyshev as C, polynomial as P
    D = 48
    j = np.arange(D + 1)
    v = np.cos(np.pi * j / D)               # chebyshev nodes in [-1, 1]
    z = np.maximum(0.25 * (v + 1.0) / 2.0, 1e-12)
    sq = np.sqrt(z)
    args = dict(W1=W1, b1=b1, W2=W2, b2=b2, W3=W3, b3=b3, W4=W4, b4=b4)
    fp = _taylor_mlp(0.5 + sq, **args)      # [5, D+1]
    fm = _taylor_mlp(0.5 - sq, **args)
    E = (fp + fm) / 2.0
    O = (fp - fm) / (2.0 * sq)
    km = np.cos(np.pi * np.outer(j, j) / D)
    wts = np.ones(D + 1)
    wts[0] = 0.5
    wts[-1] = 0.5
    out = []
    for g in (E, O):
        cc = (2.0 / D) * (g * wts[None, :]) @ km
        cc[:, 0] *= 0.5
        cc[:, -1] *= 0.5
        polys = np.zeros((_NORD, _KB))
        for o in range(_NORD):
            ch = C.Chebyshev(cc[o, :_KB])           # in v = 8z - 1
            pz = ch.convert(kind=P.Polynomial)(P.Polynomial([-1.0, 8.0]))
            polys[o, :len(pz.coef)] = pz.coef
        out.append(polys)
    return out  # E_coefs [5, 8], O_coefs [5, 8]


def _build_lhs32():
    """bf16 lhsT [32, 128] for q = k*lnz from the lnz hi/lo bf16 pair.
    Row order r = g*2 + p matches the shuffle DMA's (g, part) layout."""
    import ml_dtypes
    m = np.zeros((2 * _G, 128), np.float32)
    for g in range(_G):
        for k in range(_KB):
            m[g * 2 + 0, g * _KB + k] = float(k)
            m[g * 2 + 1, g * _KB + k] = float(k)
    return m.astype(ml_dtypes.bfloat16)


def _build_cmeo(Ec, Oc):
    """fp32 lhsT [128, 160]: block-diagonal E' = E - 0.5*O coeffs (cols
    0:80) and O coeffs (cols 80:160), output rows o-major (o*16 + g);
    the device computes out = E'(z) + x*O(z)."""
    m = np.zeros((128, 2 * _MROW), np.float32)
    for g in range(_G):
        for o in range(_NORD):
            m[g * _KB:(g + 1) * _KB, o * _G + g] = \
                (Ec[o] - 0.5 * Oc[o]).astype(np.float32)
            m[g * _KB:(g + 1) * _KB, _MROW + o * _G + g] = \
                Oc[o].astype(np.float32)
    return m


# ------------------------------------------------------------- device kernel
def _build_program():
    import concourse.bacc as bacc
    import concourse.tile as tile
    from concourse import mybir

    AluOp = mybir.AluOpType
    Act = mybir.ActivationFunctionType
    f32 = mybir.dt.float32
    bf16 = mybir.dt.bfloat16
    mmdt = getattr(mybir.dt, _MM_DTYPE)

    nc = bacc.Bacc(trn_type="TRN2", target_bir_lowering=False, debug=False,
                   num_devices=_NCORES)
    x_d = nc.declare_dram_parameter("x", [_ROWS, _F], f32, isOutput=False)
    w_d = nc.declare_dram_parameter("w", [2 * _G, 128], bf16, isOutput=False)
    c_d = nc.declare_dram_parameter("c", [128, 2 * _MROW], f32,
                                    isOutput=False)
    out_d = nc.declare_dram_parameter("out", [_NORD, _NPC], f32, isOutput=True)

    _S2 = _NST // 2           # supertile pairs (osb / xrep granularity)

    # Supertile st = (p2, sr) covers interleaved point-rows
    # r = p2*32 + g*2 + sr (g = 0..15), so that the pair-granularity
    # xrep / output DMAs have (sr, f) mergeable into one AP dim (DMA APs
    # are limited to 3 dims).

    with tile.TileContext(nc) as tc:
        with tc.tile_pool(name="consts", bufs=1) as consts, \
             tc.tile_pool(name="pre", bufs=1) as pre, \
             tc.tile_pool(name="stq", bufs=2, space="PSUM") as stq, \
             tc.tile_pool(name="sto", bufs=2, space="PSUM") as sto, \
             tc.tile_pool(name="bas", bufs=3) as bas, \
             tc.tile_pool(name="xr", bufs=2) as xr, \
             tc.tile_pool(name="tmpp", bufs=3) as tmpp, \
             tc.tile_pool(name="osbp", bufs=2) as osbp:
            w = consts.tile([2 * _G, 128], bf16)
            nc.sync.dma_start(out=w, in_=w_d[:, :])
            cmeo = consts.tile([128, 2 * _MROW], f32)
            nc.sync.dma_start(out=cmeo, in_=c_d[:, :])

            # ---- preprocessing on the [128, 512] point layout, 2 column
            # chunks: z = (x-0.5)^2, lnz = Ln(z + 1e-20); bf16 hi/lo split
            # of lnz into the column blocks of splits2.
            CF = _F // 2
            xs = pre.tile([_ROWS, _F], f32)
            sq = pre.tile([_ROWS, _F], f32)
            lnz = pre.tile([_ROWS, _F], f32)
            ltlo = pre.tile([_ROWS, _F], f32)
            splits2 = pre.tile([_ROWS, 2 * _F], bf16)
            eps = pre.tile([_ROWS, 1], f32)
            nc.vector.memset(eps[:, :], 1e-20)
            mhalf = pre.tile([_ROWS, 1], f32)
            nc.vector.memset(mhalf[:, :], -0.5)
            for c in range(2):
                cs = slice(c * CF, (c + 1) * CF)
                b0 = slice(0 * _F + c * CF, 0 * _F + (c + 1) * CF)
                b1 = slice(1 * _F + c * CF, 1 * _F + (c + 1) * CF)
                nc.sync.dma_start(out=xs[:, cs], in_=x_d[:, cs])
                nc.scalar.activation(sq[:, cs], xs[:, cs], Act.Square,
                                     bias=mhalf[:, :])
                nc.scalar.activation(lnz[:, cs], sq[:, cs], Act.Ln,
                                     bias=eps[:, :])
                nc.gpsimd.tensor_copy(splits2[:, b0], lnz[:, cs])
                nc.vector.tensor_sub(ltlo[:, cs], lnz[:, cs],
                                     splits2[:, b0])
                nc.vector.tensor_copy(splits2[:, b1], ltlo[:, cs])

            # ---- shuffle: one reshape DMA per supertile.
            # in  = splits2 rows p2*32 + g*2 + sr, iterated (g, part, f)
            # out = sh[32, 512] with row g*2 + part.
            spv = splits2[:, :].rearrange("(s2 g sr) c -> s2 sr g c",
                                          s2=_S2, g=_G, sr=2)
            sh = pre.tile([2 * _G, _NST * _F], bf16)
            for st in range(_NST):
                p2, sr = st // 2, st % 2
                nc.gpsimd.dma_start(
                    out=sh[:, st * _F:(st + 1) * _F],
                    in_=spv[p2, sr])

            # x replicated to all 5 output rows, one DMA per supertile
            # pair, read straight from DRAM: row o*16+g, col sr*512+f.
            xq = x_d.rearrange("(s2 o g sr) f -> s2 o g (sr f)",
                               o=1, g=_G, sr=2)
            outw = out_d.rearrange("o (s2 g sr f) -> o s2 g (sr f)",
                                   g=_G, sr=2, f=_F)

            for p2 in range(_S2):
                xrep = xr.tile([_MROW, 2 * _F], f32)
                nc.sync.dma_start(
                    out=xrep,
                    in_=xq[p2].to_broadcast((_NORD, _G, 2 * _F)))
                osb = osbp.tile([_MROW, 2 * _F], f32)
                for sr in range(2):
                    st = p2 * 2 + sr
                    lo = st * _F
                    hi = (st + 1) * _F
                    rel = slice(sr * _F, (sr + 1) * _F)
                    q_ps = stq.tile([128, _F], f32)
                    nc.tensor.matmul(q_ps, lhsT=w[:, :], rhs=sh[:, lo:hi],
                                     start=True, stop=True)
                    basis = bas.tile([128, _F], f32)
                    nc.scalar.activation(basis, q_ps, Act.Exp)
                    o_ps = sto.tile([_MROW, 2 * _F], f32)
                    nc.tensor.matmul(o_ps[:, 0 * _F:1 * _F],
                                     lhsT=cmeo[:, 0:_MROW].bitcast(mmdt),
                                     rhs=basis[:, :].bitcast(mmdt),
                                     start=True, stop=True)
                    nc.tensor.matmul(o_ps[:, 1 * _F:2 * _F],
                                     lhsT=cmeo[:, _MROW:2 * _MROW]
                                     .bitcast(mmdt),
                                     rhs=basis[:, :].bitcast(mmdt),
                                     start=True, stop=True)
                    tmp = tmpp.tile([_MROW, _F], f32)
                    nc.vector.tensor_mul(tmp, xrep[:, rel],
                                         o_ps[:, 1 * _F:2 * _F])
                    nc.vector.tensor_add(osb[:, rel], tmp,
                                         o_ps[:, 0 * _F:1 * _F])
                nc.sync.dma_start(out=outw[:, p2], in_=osb[:, :])

    nc.finalize()
    return nc


def _get_program():
    if "nc" not in _compiled:
        _compiled["nc"] = _build_program()
    return _compiled["nc"]


def _run(inputs, **spmd_kwargs):
    """Shard, run on 8 cores, gather. Returns (out [5, N], BassKernelResults)."""
    from concourse.bass_utils import run_bass_kernel_spmd

    x = np.ascontiguousarray(np.asarray(inputs["x"], np.float32))
    assert x.shape == (_N,), f"unexpected x shape {x.shape}"
    Ec, Oc = _fit_even_odd(inputs["W1"], inputs["b1"], inputs["W2"],
                           inputs["b2"], inputs["W3"], inputs["b3"],
                           inputs["W4"], inputs["b4"])
    wmat = _build_lhs32()
    cmeo = _build_cmeo(Ec, Oc)
    nc = _get_program()

    xs = x.reshape(_NCORES, _ROWS, _F)
    in_maps = [{"x": np.ascontiguousarray(xs[i]), "w": wmat, "c": cmeo}
               for i in range(_NCORES)]
    res = run_bass_kernel_spmd(nc, in_maps, core_ids=list(range(_NCORES)),
                               **spmd_kwargs)
    out = np.concatenate([res.results[i]["out"] for i in range(_NCORES)],
                         axis=1)
    return np.ascontiguousarray(out.astype(np.float32)), res


def kernel(**inputs):
    out, _ = _run(inputs)
    return out


if __name__ == "__main__":
    rng = np.random.default_rng(0)
    fake = {
        "x": rng.uniform(0, 1, _N).astype(np.float32),
        "W1": (rng.standard_normal((1, 15)) * 0.5).astype(np.float32),
        "b1": np.zeros(15, np.float32),
        "W2": (rng.standard_normal((15, 30)) * 0.25).astype(np.float32),
        "b2": np.zeros(30, np.float32),
        "W3": (rng.standard_normal((30, 60)) * 0.18).astype(np.float32),
        "b3": np.zeros(60, np.float32),
        "W4": (rng.standard_normal((60, 1)) * 0.13).astype(np.float32),
        "b4": np.zeros(1, np.float32),
    }
    out = kernel(**fake)
    ref = _taylor_mlp(fake["x"], fake["W1"], fake["b1"], fake["W2"],
                      fake["b2"], fake["W3"], fake["b3"], fake["W4"],
                      fake["b4"])
    for i in range(5):
        scale = np.abs(ref[i]).max()
        err = np.abs(out[i] - ref[i]).max()
        print(f"order {i}: absmax_err={err:.3e} rel={err / scale:.3e}")


# revision 19
# speedup vs baseline: 2.0569x; 1.3334x over previous
"""Trainium2 kernel for nn_CantileverPINN: MLP 1->15->30->60->1 value + first
4 derivatives w.r.t. the scalar input x at N=524288 collocation points.

Strategy: each of the 5 outputs is a smooth scalar function of x on [0,1)
(tanh-MLP composition, analytic).  Parity-fold about x=0.5: with s = x-0.5
and z = s^2 in [0, 0.25],

    f_o(x) = E_o(z) + s * O_o(z)

where E_o, O_o are degree-7 polynomials in z (equivalent to a degree-15
fit in x; host fits them from exact float64 Taylor-mode derivatives of the
MLP).  The z-monomial basis on [0, 0.25] is well conditioned here because
the Chebyshev coefficients of the outputs decay ~10x per z-degree while
monomial conversion on [0, 0.25] amplifies only ~5.8x per z-degree
(measured evaluation condition number kappa <= 3).

Device evaluation per point:
    z   = s*s + 1e-20          (DVE, exact fp32)
    lnz = Ln(z)                (ACT)
    q   = k * lnz              (PE outer product, k = 0..7 block-diagonal,
                                lnz carried as bf16 hi+lo pair -> exact to
                                ~1e-5; 16 point-groups per 128-partition
                                supertile)
    B_k = Exp(q) = z^k         (ACT)
    oE  = E^T B, oO = O^T B    (PE contraction, block-diagonal coeffs)
    out = oE + s * oO          (s replicated to all 5 outputs by a third
                                PE matmul from its bf16 hi+lo pair)

Data parallel over 8 cores: each core handles 65536 points ([128, 512]
tile); 8 supertiles of 16 point-rows each.
"""

import numpy as np

_N = 524288
_NCORES = 8
_NPC = _N // _NCORES      # 65536 points per core
_F = 512                  # free-dim columns per tile
_ROWS = _NPC // _F        # 128 point-rows per core
_G = 16                   # point-rows (groups) per supertile
_KB = 8                   # z-monomial terms per group (degree 7 in z)
_NST = _ROWS // _G        # 8 supertiles
_NORD = 5                 # outputs: w, w_x, w_xx, w_xxx, w_xxxx
_MROW = _NORD * _G        # 80 output rows per supertile (o-major)

_MM_DTYPE = "float32r"    # contraction matmul dtype: "float32" | "float32r"

_compiled = {}


# ----------------------------------------------------------------- host math
def _taylor_mlp(x, W1, b1, W2, b2, W3, b3, W4, b4):
    """Exact value + derivatives (orders 0..4) of the MLP at points x.

    float64 throughout; returns [5, n]."""
    x = np.asarray(x, np.float64)
    n = x.shape[0]
    W1, b1, W2, b2, W3, b3, W4, b4 = [
        np.asarray(a, np.float64) for a in (W1, b1, W2, b2, W3, b3, W4, b4)
    ]
    w1 = W1[0]
    a0 = x[:, None] * w1[None, :] + b1[None, :]
    a1 = np.broadcast_to(w1[None, :], (n, w1.shape[0])).copy()
    a2 = np.zeros_like(a0)
    a3 = np.zeros_like(a0)
    a4 = np.zeros_like(a0)

    def tanh_chain(a0, a1, a2, a3, a4):
        t = np.tanh(a0)
        u = 1.0 - t * t
        s2 = -2.0 * t * u
        s3 = u * (6.0 * t * t - 2.0)
        s4 = 8.0 * t * u * (2.0 - 3.0 * t * t)
        h0 = t
        h1 = u * a1
        h2 = s2 * a1**2 + u * a2
        h3 = s3 * a1**3 + 3.0 * s2 * a1 * a2 + u * a3
        h4 = (s4 * a1**4 + 6.0 * s3 * a1**2 * a2
              + s2 * (3.0 * a2**2 + 4.0 * a1 * a3) + u * a4)
        return h0, h1, h2, h3, h4

    for W, b in ((W2, b2), (W3, b3)):
        h = tanh_chain(a0, a1, a2, a3, a4)
        a0 = h[0] @ W + b[None, :]
        a1 = h[1] @ W
        a2 = h[2] @ W
        a3 = h[3] @ W
        a4 = h[4] @ W
    h = tanh_chain(a0, a1, a2, a3, a4)
    return np.stack([(h[i] @ W4)[:, 0] + (b4[0] if i == 0 else 0.0)
                     for i in range(5)])


def _fit_even_odd(W1, b1, W2, b2, W3, b3, W4, b4):
    """Monomial coefficients [5, _KB] of E_o(z), O_o(z) on z in [0, 0.25]
    where f_o(0.5 + s) = E_o(s^2) + s * O_o(s^2)."""
    from numpy.polynomial import chebyshev as C, polynomial as P
    D = 48
    j = np.arange(D + 1)
    v = np.cos(np.pi * j / D)               # chebyshev nodes in [-1, 1]
    z = np.maximum(0.25 * (v + 1.0) / 2.0, 1e-12)
    sq = np.sqrt(z)
    args = dict(W1=W1, b1=b1, W2=W2, b2=b2, W3=W3, b3=b3, W4=W4, b4=b4)
    fp = _taylor_mlp(0.5 + sq, **args)      # [5, D+1]
    fm = _taylor_mlp(0.5 - sq, **args)
    E = (fp + fm) / 2.0
    O = (fp - fm) / (2.0 * sq)
    km = np.cos(np.pi * np.outer(j, j) / D)
    wts = np.ones(D + 1)
    wts[0] = 0.5
    wts[-1] = 0.5
    out = []
    for g in (E, O):
        cc = (2.0 / D) * (g * wts[None, :]) @ km
        cc[:, 0] *= 0.5
        cc[:, -1] *= 0.5
        polys = np.zeros((_NORD, _KB))
        for o in range(_NORD):
            ch = C.Chebyshev(cc[o, :_KB])           # in v = 8z - 1
            pz = ch.convert(kind=P.Polynomial)(P.Polynomial([-1.0, 8.0]))
            polys[o, :len(pz.coef)] = pz.coef
        out.append(polys)
    return out  # E_coefs [5, 8], O_coefs [5, 8]


def _build_lhs32():
    """bf16 lhsT [32, 128] for q = k*lnz from the lnz hi/lo bf16 pair.
    Row order r = g*2 + p matches the shuffle DMA's (g, part) layout."""
    import ml_dtypes
    m = np.zeros((2 * _G, 128), np.float32)
    for g in range(_G):
        for k in range(_KB):
            m[g * 2 + 0, g * _KB + k] = float(k)
            m[g * 2 + 1, g * _KB + k] = float(k)
    return m.astype(ml_dtypes.bfloat16)


def _build_cmeo(Ec, Oc):
    """fp32 lhsT [128, 160]: block-diagonal E' = E - 0.5*O coeffs (cols
    0:80) and O coeffs (cols 80:160), output rows g-major (g*5 + o) so
    output/xrep DMA descriptors split 16-way across the DMA engines;
    the device computes out = E'(z) + x*O(z)."""
    m = np.zeros((128, 2 * _MROW), np.float32)
    for g in range(_G):
        for o in range(_NORD):
            m[g * _KB:(g + 1) * _KB, g * _NORD + o] = \
                (Ec[o] - 0.5 * Oc[o]).astype(np.float32)
            m[g * _KB:(g + 1) * _KB, _MROW + g * _NORD + o] = \
                Oc[o].astype(np.float32)
    return m


# ------------------------------------------------------------- device kernel
def _build_program():
    import concourse.bacc as bacc
    import concourse.tile as tile
    from concourse import mybir

    AluOp = mybir.AluOpType
    Act = mybir.ActivationFunctionType
    f32 = mybir.dt.float32
    bf16 = mybir.dt.bfloat16
    mmdt = getattr(mybir.dt, _MM_DTYPE)

    nc = bacc.Bacc(trn_type="TRN2", target_bir_lowering=False, debug=False,
                   num_devices=_NCORES)
    x_d = nc.declare_dram_parameter("x", [_ROWS, _F], f32, isOutput=False)
    w_d = nc.declare_dram_parameter("w", [2 * _G, 128], bf16, isOutput=False)
    c_d = nc.declare_dram_parameter("c", [128, 2 * _MROW], f32,
                                    isOutput=False)
    out_d = nc.declare_dram_parameter("out", [_NORD, _NPC], f32, isOutput=True)

    _NQ = _NST // 4           # supertile quads (osb / xrep granularity)

    # Supertile st = (p2, sr) covers interleaved point-rows
    # r = q*64 + g*4 + i  with  q = st // 4, i = st % 4  (g = 0..15),
    # so the quad-granularity xrep / output DMAs have (i, f) mergeable
    # into one AP dim (DMA APs are limited to 3 dims) and descriptors
    # split 16-way (outer dim g) across the DMA engines.

    with tile.TileContext(nc) as tc:
        with tc.tile_pool(name="consts", bufs=1) as consts, \
             tc.tile_pool(name="pre", bufs=1) as pre, \
             tc.tile_pool(name="stq", bufs=2, space="PSUM") as stq, \
             tc.tile_pool(name="sto", bufs=1, space="PSUM") as sto, \
             tc.tile_pool(name="ste", bufs=2, space="PSUM") as ste, \
             tc.tile_pool(name="bas", bufs=3) as bas, \
             tc.tile_pool(name="xr", bufs=2) as xr, \
             tc.tile_pool(name="tmpp", bufs=3) as tmpp, \
             tc.tile_pool(name="osbp", bufs=2) as osbp:
            w = consts.tile([2 * _G, 128], bf16)
            nc.sync.dma_start(out=w, in_=w_d[:, :])
            cmeo = consts.tile([128, 2 * _MROW], mmdt)
            nc.sync.dma_start(out=cmeo, in_=c_d[:, :].bitcast(mmdt))

            # ---- preprocessing on the [128, 512] point layout, 2 column
            # chunks: z = (x-0.5)^2, lnz = Ln(z + 1e-20); bf16 hi/lo split
            # of lnz into the column blocks of splits2.  Scalar engine
            # only ever needs the Ln and Exp tables (no table ping-pong).
            CF = _F // 2
            xs = pre.tile([_ROWS, _F], f32)
            s32 = pre.tile([_ROWS, _F], f32)
            sq = pre.tile([_ROWS, _F], f32)
            lnz = pre.tile([_ROWS, _F], f32)
            ltlo = pre.tile([_ROWS, _F], f32)
            splits2 = pre.tile([_ROWS, 2 * _F], bf16)
            eps = pre.tile([_ROWS, 1], f32)
            nc.vector.memset(eps[:, :], 1e-20)
            for c in range(2):
                cs = slice(c * CF, (c + 1) * CF)
                b0 = slice(0 * _F + c * CF, 0 * _F + (c + 1) * CF)
                b1 = slice(1 * _F + c * CF, 1 * _F + (c + 1) * CF)
                nc.sync.dma_start(out=xs[:, cs], in_=x_d[:, cs])
                nc.vector.tensor_scalar_add(s32[:, cs], xs[:, cs], -0.5)
                nc.gpsimd.tensor_mul(sq[:, cs], s32[:, cs], s32[:, cs])
                nc.scalar.activation(lnz[:, cs], sq[:, cs], Act.Ln,
                                     bias=eps[:, :])
                nc.vector.tensor_copy(splits2[:, b0], lnz[:, cs])
                nc.vector.tensor_sub(ltlo[:, cs], lnz[:, cs],
                                     splits2[:, b0])
                nc.vector.tensor_copy(splits2[:, b1], ltlo[:, cs])

            # ---- shuffle: one reshape DMA per supertile.
            # in  = splits2 rows q*64 + g*4 + i, iterated (g, part, f)
            # out = sh[32, 512] with row g*2 + part.
            spv = splits2[:, :].rearrange("(q g i) c -> q i g c",
                                          q=_NQ, g=_G, i=4)
            sh = pre.tile([2 * _G, _NST * _F], bf16)
            for st in range(_NST):
                nc.gpsimd.dma_start(
                    out=sh[:, st * _F:(st + 1) * _F],
                    in_=spv[st // 4, st % 4])

            # x replicated to all 5 output rows (row g*5 + o), one DMA
            # per supertile quad, read straight from DRAM.
            xq = x_d.rearrange("(q g o i) f -> q g o (i f)",
                               g=_G, o=1, i=4)
            outw = out_d.rearrange("o (q g i f) -> q g o (i f)",
                                   g=_G, i=4, f=_F)

            for q in range(_NQ):
                xrep = xr.tile([_MROW, 4 * _F], f32)
                nc.gpsimd.dma_start(
                    out=xrep,
                    in_=xq[q].to_broadcast((_G, _NORD, 4 * _F)))
                osb = osbp.tile([_MROW, 4 * _F], f32)
                for h in range(2):      # supertile pairs within the quad
                    bases = []
                    for sr in range(2):
                        st = q * 4 + h * 2 + sr
                        q_ps = stq.tile([128, _F], f32)
                        nc.tensor.matmul(
                            q_ps, lhsT=w[:, :],
                            rhs=sh[:, st * _F:(st + 1) * _F],
                            start=True, stop=True)
                        basis = bas.tile([128, _F], mmdt)
                        nc.scalar.activation(basis, q_ps, Act.Exp)
                        bases.append(basis)
                    o_ps = sto.tile([_MROW, 2 * _F], f32)
                    for sr in range(2):
                        bs = bases[sr][:, :]
                        nc.tensor.matmul(o_ps[:, sr * _F:(sr + 1) * _F],
                                         lhsT=cmeo[:, _MROW:2 * _MROW],
                                         rhs=bs, start=True, stop=True)
                    tmp = tmpp.tile([_MROW, 2 * _F], f32)
                    rel2 = slice(h * 2 * _F, (h + 1) * 2 * _F)
                    nc.vector.tensor_mul(tmp, xrep[:, rel2], o_ps)
                    o_ps2 = ste.tile([_MROW, 2 * _F], f32)
                    for sr in range(2):
                        bs = bases[sr][:, :]
                        nc.tensor.matmul(o_ps2[:, sr * _F:(sr + 1) * _F],
                                         lhsT=cmeo[:, 0:_MROW],
                                         rhs=bs, start=True, stop=True)
                    nc.vector.tensor_add(osb[:, rel2], tmp, o_ps2)
                nc.sync.dma_start(out=outw[q], in_=osb[:, :])

    nc.finalize()
    return nc


def _get_program():
    if "nc" not in _compiled:
        _compiled["nc"] = _build_program()
    return _compiled["nc"]


def _run(inputs, **spmd_kwargs):
    """Shard, run on 8 cores, gather. Returns (out [5, N], BassKernelResults)."""
    from concourse.bass_utils import run_bass_kernel_spmd

    x = np.ascontiguousarray(np.asarray(inputs["x"], np.float32))
    assert x.shape == (_N,), f"unexpected x shape {x.shape}"
    Ec, Oc = _fit_even_odd(inputs["W1"], inputs["b1"], inputs["W2"],
                           inputs["b2"], inputs["W3"], inputs["b3"],
                           inputs["W4"], inputs["b4"])
    wmat = _build_lhs32()
    cmeo = _build_cmeo(Ec, Oc)
    nc = _get_program()

    xs = x.reshape(_NCORES, _ROWS, _F)
    in_maps = [{"x": np.ascontiguousarray(xs[i]), "w": wmat, "c": cmeo}
               for i in range(_NCORES)]
    res = run_bass_kernel_spmd(nc, in_maps, core_ids=list(range(_NCORES)),
                               **spmd_kwargs)
    out = np.concatenate([res.results[i]["out"] for i in range(_NCORES)],
                         axis=1)
    return np.ascontiguousarray(out.astype(np.float32)), res


def kernel(**inputs):
    out, _ = _run(inputs)
    return out


if __name__ == "__main__":
    rng = np.random.default_rng(0)
    fake = {
        "x": rng.uniform(0, 1, _N).astype(np.float32),
        "W1": (rng.standard_normal((1, 15)) * 0.5).astype(np.float32),
        "b1": np.zeros(15, np.float32),
        "W2": (rng.standard_normal((15, 30)) * 0.25).astype(np.float32),
        "b2": np.zeros(30, np.float32),
        "W3": (rng.standard_normal((30, 60)) * 0.18).astype(np.float32),
        "b3": np.zeros(60, np.float32),
        "W4": (rng.standard_normal((60, 1)) * 0.13).astype(np.float32),
        "b4": np.zeros(1, np.float32),
    }
    out = kernel(**fake)
    ref = _taylor_mlp(fake["x"], fake["W1"], fake["b1"], fake["W2"],
                      fake["b2"], fake["W3"], fake["b3"], fake["W4"],
                      fake["b4"])
    for i in range(5):
        scale = np.abs(ref[i]).max()
        err = np.abs(out[i] - ref[i]).max()
        print(f"order {i}: absmax_err={err:.3e} rel={err / scale:.3e}")


# revision 25
# speedup vs baseline: 2.4310x; 1.1819x over previous
"""Trainium2 kernel for nn_CantileverPINN: MLP 1->15->30->60->1 value + first
4 derivatives w.r.t. the scalar input x at N=524288 collocation points.

Strategy: each of the 5 outputs is a smooth scalar function of x on [0,1)
(tanh-MLP composition, analytic).  Parity-fold about x=0.5: with s = x-0.5
and z = s^2 in [0, 0.25],

    f_o(x) = E_o(z) + s * O_o(z)

where E_o, O_o are degree-7 polynomials in z (equivalent to a degree-15
fit in x; host fits them from exact float64 Taylor-mode derivatives of the
MLP).  The z-monomial basis on [0, 0.25] is well conditioned here because
the Chebyshev coefficients of the outputs decay ~10x per z-degree while
monomial conversion on [0, 0.25] amplifies only ~5.8x per z-degree
(measured evaluation condition number kappa <= 3).

Device evaluation per point:
    z   = s*s + 1e-20          (DVE, exact fp32)
    lnz = Ln(z)                (ACT)
    q   = k * lnz              (PE outer product, k = 0..7 block-diagonal,
                                lnz carried as bf16 hi+lo pair -> exact to
                                ~1e-5; 16 point-groups per 128-partition
                                supertile)
    B_k = Exp(q) = z^k         (ACT)
    oE  = E^T B, oO = O^T B    (PE contraction, block-diagonal coeffs)
    out = oE + s * oO          (s replicated to all 5 outputs by a third
                                PE matmul from its bf16 hi+lo pair)

Data parallel over 8 cores: each core handles 65536 points ([128, 512]
tile); 8 supertiles of 16 point-rows each.
"""

import numpy as np

_N = 524288
_NCORES = 8
_NPC = _N // _NCORES      # 65536 points per core
_F = 512                  # free-dim columns per tile
_ROWS = _NPC // _F        # 128 point-rows per core
_G = 16                   # point-rows (groups) per supertile
_KB = 8                   # z-monomial terms per group (degree 7 in z)
_NST = _ROWS // _G        # 8 supertiles
_NORD = 5                 # outputs: w, w_x, w_xx, w_xxx, w_xxxx
_MROW = _NORD * _G        # 80 output rows per supertile (o-major)

_MM_DTYPE = "float32r"    # contraction matmul dtype: "float32" | "float32r"

_compiled = {}


# ----------------------------------------------------------------- host math
def _taylor_mlp(x, W1, b1, W2, b2, W3, b3, W4, b4):
    """Exact value + derivatives (orders 0..4) of the MLP at points x.

    float64 throughout; returns [5, n]."""
    x = np.asarray(x, np.float64)
    n = x.shape[0]
    W1, b1, W2, b2, W3, b3, W4, b4 = [
        np.asarray(a, np.float64) for a in (W1, b1, W2, b2, W3, b3, W4, b4)
    ]
    w1 = W1[0]
    a0 = x[:, None] * w1[None, :] + b1[None, :]
    a1 = np.broadcast_to(w1[None, :], (n, w1.shape[0])).copy()
    a2 = np.zeros_like(a0)
    a3 = np.zeros_like(a0)
    a4 = np.zeros_like(a0)

    def tanh_chain(a0, a1, a2, a3, a4):
        t = np.tanh(a0)
        u = 1.0 - t * t
        s2 = -2.0 * t * u
        s3 = u * (6.0 * t * t - 2.0)
        s4 = 8.0 * t * u * (2.0 - 3.0 * t * t)
        h0 = t
        h1 = u * a1
        h2 = s2 * a1**2 + u * a2
        h3 = s3 * a1**3 + 3.0 * s2 * a1 * a2 + u * a3
        h4 = (s4 * a1**4 + 6.0 * s3 * a1**2 * a2
              + s2 * (3.0 * a2**2 + 4.0 * a1 * a3) + u * a4)
        return h0, h1, h2, h3, h4

    for W, b in ((W2, b2), (W3, b3)):
        h = tanh_chain(a0, a1, a2, a3, a4)
        a0 = h[0] @ W + b[None, :]
        a1 = h[1] @ W
        a2 = h[2] @ W
        a3 = h[3] @ W
        a4 = h[4] @ W
    h = tanh_chain(a0, a1, a2, a3, a4)
    return np.stack([(h[i] @ W4)[:, 0] + (b4[0] if i == 0 else 0.0)
                     for i in range(5)])


def _fit_even_odd(W1, b1, W2, b2, W3, b3, W4, b4):
    """Monomial coefficients [5, _KB] of E_o(z), O_o(z) on z in [0, 0.25]
    where f_o(0.5 + s) = E_o(s^2) + s * O_o(s^2)."""
    from numpy.polynomial import chebyshev as C, polynomial as P
    D = 48
    j = np.arange(D + 1)
    v = np.cos(np.pi * j / D)               # chebyshev nodes in [-1, 1]
    z = np.maximum(0.25 * (v + 1.0) / 2.0, 1e-12)
    sq = np.sqrt(z)
    args = dict(W1=W1, b1=b1, W2=W2, b2=b2, W3=W3, b3=b3, W4=W4, b4=b4)
    fp = _taylor_mlp(0.5 + sq, **args)      # [5, D+1]
    fm = _taylor_mlp(0.5 - sq, **args)
    E = (fp + fm) / 2.0
    O = (fp - fm) / (2.0 * sq)
    km = np.cos(np.pi * np.outer(j, j) / D)
    wts = np.ones(D + 1)
    wts[0] = 0.5
    wts[-1] = 0.5
    out = []
    for g in (E, O):
        cc = (2.0 / D) * (g * wts[None, :]) @ km
        cc[:, 0] *= 0.5
        cc[:, -1] *= 0.5
        polys = np.zeros((_NORD, _KB))
        for o in range(_NORD):
            ch = C.Chebyshev(cc[o, :_KB])           # in v = 8z - 1
            pz = ch.convert(kind=P.Polynomial)(P.Polynomial([-1.0, 8.0]))
            polys[o, :len(pz.coef)] = pz.coef
        out.append(polys)
    return out  # E_coefs [5, 8], O_coefs [5, 8]


def _build_lhs32():
    """bf16 lhsT [32, 128] for q = k*lnz from the lnz hi/lo bf16 pair.
    Row order r = g*2 + p matches the shuffle DMA's (g, part) layout."""
    import ml_dtypes
    m = np.zeros((2 * _G, 128), np.float32)
    for g in range(_G):
        for k in range(_KB):
            m[g * 2 + 0, g * _KB + k] = float(k)
            m[g * 2 + 1, g * _KB + k] = float(k)
    return m.astype(ml_dtypes.bfloat16)


def _build_cmeo(Ec, Oc):
    """fp32 lhsT [128, 160]: block-diagonal E' = E - 0.5*O coeffs (cols
    0:80) and O coeffs (cols 80:160), output rows g-major (g*5 + o) so
    output/xrep DMA descriptors split 16-way across the DMA engines;
    the device computes out = E'(z) + x*O(z)."""
    m = np.zeros((128, 2 * _MROW), np.float32)
    for g in range(_G):
        for o in range(_NORD):
            m[g * _KB:(g + 1) * _KB, g * _NORD + o] = \
                (Ec[o] - 0.5 * Oc[o]).astype(np.float32)
            m[g * _KB:(g + 1) * _KB, _MROW + g * _NORD + o] = \
                Oc[o].astype(np.float32)
    return m


# ------------------------------------------------------------- device kernel
def _build_program():
    import concourse.bacc as bacc
    import concourse.tile as tile
    from concourse import mybir

    AluOp = mybir.AluOpType
    Act = mybir.ActivationFunctionType
    f32 = mybir.dt.float32
    bf16 = mybir.dt.bfloat16
    mmdt = getattr(mybir.dt, _MM_DTYPE)

    nc = bacc.Bacc(trn_type="TRN2", target_bir_lowering=False, debug=False,
                   num_devices=_NCORES)
    x_d = nc.declare_dram_parameter("x", [_ROWS, _F], f32, isOutput=False)
    w_d = nc.declare_dram_parameter("w", [2 * _G, 128], bf16, isOutput=False)
    c_d = nc.declare_dram_parameter("c", [128, 2 * _MROW], f32,
                                    isOutput=False)
    out_d = nc.declare_dram_parameter("out", [_NORD, _NPC], f32, isOutput=True)

    _NQ = _NST // 4           # supertile quads (osb / xrep granularity)

    # Supertile st = (p2, sr) covers interleaved point-rows
    # r = q*64 + g*4 + i  with  q = st // 4, i = st % 4  (g = 0..15),
    # so the quad-granularity xrep / output DMAs have (i, f) mergeable
    # into one AP dim (DMA APs are limited to 3 dims) and descriptors
    # split 16-way (outer dim g) across the DMA engines.

    with tile.TileContext(nc) as tc:
        with tc.tile_pool(name="consts", bufs=1) as consts, \
             tc.tile_pool(name="pre", bufs=1) as pre, \
             tc.tile_pool(name="stq", bufs=2, space="PSUM") as stq, \
             tc.tile_pool(name="sto", bufs=2, space="PSUM") as sto, \
             tc.tile_pool(name="ste", bufs=1, space="PSUM") as ste, \
             tc.tile_pool(name="bas", bufs=3) as bas, \
             tc.tile_pool(name="xr", bufs=2) as xr, \
             tc.tile_pool(name="tmpp", bufs=3) as tmpp, \
             tc.tile_pool(name="osbp", bufs=2) as osbp:
            # x first in the sync DMA queue so the upfront compute is not
            # starved behind bulk transfers (same-queue DMAs run FIFO).
            xs0 = pre.tile([_ROWS, _F], f32)
            nc.sync.dma_start(out=xs0, in_=x_d[:, :])
            w = consts.tile([2 * _G, 128], bf16)
            nc.sync.dma_start(out=w, in_=w_d[:, :])
            cmeo = consts.tile([128, 2 * _MROW], mmdt)
            nc.sync.dma_start(out=cmeo, in_=c_d[:, :].bitcast(mmdt))

            # ---- preprocessing on the [128, 512] point layout, 2 column
            # chunks: z = (x-0.5)^2, lnz = Ln(z + 1e-20); bf16 hi/lo split
            # of lnz into the column blocks of splits2.  Scalar engine
            # only ever needs the Ln and Exp tables (no table ping-pong).
            CF = _F // 2
            s32 = pre.tile([_ROWS, _F], f32)
            sq = pre.tile([_ROWS, _F], f32)
            lnz = pre.tile([_ROWS, _F], f32)
            ltlo = pre.tile([_ROWS, _F], f32)
            splits2 = pre.tile([_ROWS, 2 * _F], bf16)
            eps = pre.tile([_ROWS, 1], f32)
            nc.vector.memset(eps[:, :], 1e-20)
            for c in range(2):
                cs = slice(c * CF, (c + 1) * CF)
                b0 = slice(0 * _F + c * CF, 0 * _F + (c + 1) * CF)
                b1 = slice(1 * _F + c * CF, 1 * _F + (c + 1) * CF)
                nc.vector.tensor_scalar_add(s32[:, cs], xs0[:, cs], -0.5)
                nc.gpsimd.tensor_mul(sq[:, cs], s32[:, cs], s32[:, cs])
                nc.scalar.activation(lnz[:, cs], sq[:, cs], Act.Ln,
                                     bias=eps[:, :])
                nc.vector.tensor_copy(splits2[:, b0], lnz[:, cs])
                nc.vector.tensor_sub(ltlo[:, cs], lnz[:, cs],
                                     splits2[:, b0])
                nc.vector.tensor_copy(splits2[:, b1], ltlo[:, cs])

            # ---- shuffle: one reshape DMA per supertile.
            # in  = splits2 rows q*64 + g*4 + i, iterated (g, part, f)
            # out = sh[32, 512] with row g*2 + part.
            spv = splits2[:, :].rearrange("(q g i) c -> q i g c",
                                          q=_NQ, g=_G, i=4)
            sh = pre.tile([2 * _G, _NST * _F], bf16)
            for st in range(_NST):
                nc.gpsimd.dma_start(
                    out=sh[:, st * _F:(st + 1) * _F],
                    in_=spv[st // 4, st % 4])

            # x replicated to all 5 output rows (row g*5 + o), one DMA
            # per supertile quad, read straight from DRAM.
            xq = x_d.rearrange("(q g o i) f -> q g o (i f)",
                               g=_G, o=1, i=4)
            outw = out_d.rearrange("o (q g i f) -> q g o (i f)",
                                   g=_G, i=4, f=_F)

            # xrep q=0 rides the sync FIFO behind the consts (so it cannot
            # starve the x load); q=1 rides the gpsimd FIFO behind the
            # shuffles.
            xreps = []
            for q in range(_NQ):
                xrep = xr.tile([_MROW, 4 * _F], f32)
                eng = nc.sync if q == 0 else nc.gpsimd
                eng.dma_start(
                    out=xrep,
                    in_=xq[q].to_broadcast((_G, _NORD, 4 * _F)))
                xreps.append(xrep)

            for q in range(_NQ):
                xrep = xreps[q]
                osb = osbp.tile([_MROW, 4 * _F], f32)
                for h in range(2):      # supertile pairs within the quad
                    bases = []
                    for sr in range(2):
                        st = q * 4 + h * 2 + sr
                        q_ps = stq.tile([128, _F], f32)
                        nc.tensor.matmul(
                            q_ps, lhsT=w[:, :],
                            rhs=sh[:, st * _F:(st + 1) * _F],
                            start=True, stop=True)
                        basis = bas.tile([128, _F], mmdt)
                        nc.scalar.activation(basis, q_ps, Act.Exp)
                        bases.append(basis)
                    o_ps = sto.tile([_MROW, 2 * _F], f32)
                    for sr in range(2):
                        bs = bases[sr][:, :]
                        nc.tensor.matmul(o_ps[:, sr * _F:(sr + 1) * _F],
                                         lhsT=cmeo[:, _MROW:2 * _MROW],
                                         rhs=bs, start=True, stop=True)
                    tmp = tmpp.tile([_MROW, 2 * _F], f32)
                    rel2 = slice(h * 2 * _F, (h + 1) * 2 * _F)
                    nc.vector.tensor_mul(tmp, xrep[:, rel2], o_ps)
                    o_ps2 = ste.tile([_MROW, 2 * _F], f32)
                    for sr in range(2):
                        bs = bases[sr][:, :]
                        nc.tensor.matmul(o_ps2[:, sr * _F:(sr + 1) * _F],
                                         lhsT=cmeo[:, 0:_MROW],
                                         rhs=bs, start=True, stop=True)
                    nc.vector.tensor_add(osb[:, rel2], tmp, o_ps2)
                    nc.sync.dma_start(
                        out=outw[q][:, :, rel2],
                        in_=osb[:, rel2])

    nc.finalize()
    return nc


def _get_program():
    if "nc" not in _compiled:
        _compiled["nc"] = _build_program()
    return _compiled["nc"]


def _run(inputs, **spmd_kwargs):
    """Shard, run on 8 cores, gather. Returns (out [5, N], BassKernelResults)."""
    from concourse.bass_utils import run_bass_kernel_spmd

    x = np.ascontiguousarray(np.asarray(inputs["x"], np.float32))
    assert x.shape == (_N,), f"unexpected x shape {x.shape}"
    Ec, Oc = _fit_even_odd(inputs["W1"], inputs["b1"], inputs["W2"],
                           inputs["b2"], inputs["W3"], inputs["b3"],
                           inputs["W4"], inputs["b4"])
    wmat = _build_lhs32()
    cmeo = _build_cmeo(Ec, Oc)
    nc = _get_program()

    xs = x.reshape(_NCORES, _ROWS, _F)
    in_maps = [{"x": np.ascontiguousarray(xs[i]), "w": wmat, "c": cmeo}
               for i in range(_NCORES)]
    res = run_bass_kernel_spmd(nc, in_maps, core_ids=list(range(_NCORES)),
                               **spmd_kwargs)
    out = np.concatenate([res.results[i]["out"] for i in range(_NCORES)],
                         axis=1)
    return np.ascontiguousarray(out.astype(np.float32)), res


def kernel(**inputs):
    out, _ = _run(inputs)
    return out


if __name__ == "__main__":
    rng = np.random.default_rng(0)
    fake = {
        "x": rng.uniform(0, 1, _N).astype(np.float32),
        "W1": (rng.standard_normal((1, 15)) * 0.5).astype(np.float32),
        "b1": np.zeros(15, np.float32),
        "W2": (rng.standard_normal((15, 30)) * 0.25).astype(np.float32),
        "b2": np.zeros(30, np.float32),
        "W3": (rng.standard_normal((30, 60)) * 0.18).astype(np.float32),
        "b3": np.zeros(60, np.float32),
        "W4": (rng.standard_normal((60, 1)) * 0.13).astype(np.float32),
        "b4": np.zeros(1, np.float32),
    }
    out = kernel(**fake)
    ref = _taylor_mlp(fake["x"], fake["W1"], fake["b1"], fake["W2"],
                      fake["b2"], fake["W3"], fake["b3"], fake["W4"],
                      fake["b4"])
    for i in range(5):
        scale = np.abs(ref[i]).max()
        err = np.abs(out[i] - ref[i]).max()
        print(f"order {i}: absmax_err={err:.3e} rel={err / scale:.3e}")
